# revision 52
# baseline (speedup 1.0000x reference)
"""Multi-head causal attention with RoPE (B=1, S=4096, D=1024, H=16) on 8
Trainium2 NeuronCores.

Sharding: tensor-parallel over heads - each core computes 2 heads (QKV
projections column-sliced, attention, and its rank-128 partial of the output
projection; host sums the 8 partials = row-parallel wo).

Design (v4):
  - QKV projections in hi/lo fp8 DoubleRow form: x = xh(e4m3)+xl(e5m2) and
    w*32 = wh(e4m3)+wl(e5m2) host-side; (wh.xh + wl.xh + wh.xl) via three
    DoubleRow groups (K=256/instr, 0.5 cyc/row); v is computed already
    transposed ([seq, head_dim]) by swapping matmul operands.
  - RoPE without swap-projections: DVE muls by host-prepped cos/sin rows, a
    partition-shift SBUF DMA builds the pair-partner tensor, and the final
    SBUF-only add runs on the otherwise-idle Pool (GpSimd) engine.
  - scores in fp16 at 1 cyc/row; above-diagonal 128x512 tiles are skipped
    and diagonal tiles narrowed to their valid query range.
  - attnV in fp8 DoubleRow at 0.5 cyc/row: lhsT pairs (v_hi e4m3, v_lo e5m2)
    against a stride-0-duplicated fp8 ex rhs - v at ~bf16 precision, half
    the PE cost. ex for non-diagonal tiles is e4m3: on DVE one
    tensor_scalar (y*2^-7 + 20) with round+saturate into uint8 bits
    (negatives saturate to 0 = e4m3 +0.0); on ACT a native exp with output
    cast to f8e4 (bias matches the bit-trick's mean ratio). Diagonal tiles
    keep the fp16 Schraudolph trick with the 0/31743 mask-folding gate and
    multiply a separate f16 v copy pre-scaled by 2^-4.5 so both paths land
    on the same absolute scale (e4 trick value = 2^((bits-56)/8)).
  - denominators from a ones column in the v tiles; reciprocal rows are
    partition-broadcast by DMA (free-dim stride-0 read) and the normalize
    muls read the attnV PSUM directly; the h1 accumulator lives at
    partitions 63:128 (ones row first) so afin[64:128] is written in place
    with no partition-move DMA; o-proj is a single K=128 fp16 matmul per
    128-dout tile.
  - scheduling: per-chunk epilogue (normalize+o-proj) is deferred past the
    NEXT chunk's projections; exp ops and PSUM-evacuation copies are
    round-robined over ACT and DVE; next-chunk x/cos DMAs prefetch behind
    the latency-critical shift DMAs; the last chunk writes its output in
    per-tile DMAs to shorten the drain.
"""
import math
import numpy as np

import concourse.bass as bass
import concourse.mybir as mybir
import concourse.tile as tile
from concourse.bass_utils import run_bass_kernel_spmd
from concourse.alu_op_type import AluOpType

B, S, D, H = 1, 4096, 1024, 16
HD = D // H            # 64
NC = 8                 # cores
HPC = H // NC          # 2 heads per core
SQC = 512              # seq chunk (matmul free dim)
NJ = S // SQC          # 8 chunks
NKT = S // 128         # 32 sk partition tiles
KT = D // 128          # 8 contraction tiles for projections

F32 = mybir.dt.float32
F16 = mybir.dt.float16
BF16 = mybir.dt.bfloat16
I16 = mybir.dt.int16
U8 = mybir.dt.uint8
F8E4 = mybir.dt.float8e4
F8E5 = mybir.dt.float8e5
DRMODE = mybir.MatmulPerfMode.DoubleRow
AF = mybir.ActivationFunctionType

L2E1024 = math.log2(math.e) * 1024.0          # 1477.3197 (y = z * this)
CS = math.sqrt(L2E1024 / 8.0)                 # 13.5891 folded into cos/sin
EBIAS = 15360.0                               # fp16 exponent bias << 10
ECLAMP = 31743.0                              # just below fp16 +inf bits
ACT_SCALE = 1.0 / L2E1024
RMEAN = 1.0406936                             # bit-trick mean ratio
# e4m3 trick: bits = z*8*log2e + BIAS8 = y/128 + BIAS8 (uint8, sat at 0)
BIAS8 = 20.0
SC8 = 2.0 ** ((BIAS8 - 56.0) / 8.0)           # e4 trick global scale 2^-4.5
ACT_BIAS8 = math.log(RMEAN * SC8)             # ACT exp out matches trick
# diag f16 path value = r*e^z (scale 1); bridge via v16 pre-scale SC8

_MAX_WAITS = 1


def _fix_waits(nc):
    """walrus in this container rejects >1 sync-wait per instruction
    ("Too many sync wait commands"); split excess waits onto preceding
    same-engine NoOps (engine blocks in order, semantics preserved)."""
    n = 0
    for fn in nc.m.functions:
        for bb in fn.blocks:
            new_list = []
            for inst in bb.instructions:
                si = getattr(inst, "sync_info", None)
                if si is not None and si.on_wait and len(si.on_wait) > _MAX_WAITS:
                    waits = list(si.on_wait)
                    excess, keep = waits[:-_MAX_WAITS], waits[-_MAX_WAITS:]
                    for j in range(0, len(excess), _MAX_WAITS):
                        nop = mybir.InstNoOp(
                            name=f"I-waitfix-{nc.next_id()}",
                            ins=[],
                            outs=[],
                            engine=inst.engine,
                            sync_info=mybir.SyncInfo(
                                on_wait=excess[j : j + _MAX_WAITS], on_update=[]
                            ),
                        )
                        nc.register_instruction(nop)
                        new_list.append(nop)
                        n += 1
                    si.on_wait = keep
                new_list.append(inst)
            bb.instructions[:] = new_list
    return n


def build_program(mode: str, opts=None):
    """mode: 'causal' (skip above-diag tiles; gate tiles fold the mask),
    'zeros' (no mask, full attention), 'general' (additive mask, full)."""
    causal = mode == "causal"
    o = {
        # non-diag exp engine shares: ACT (f8 native exp) / DVE (u8 trick)
        "exp_w": (3, 2),
        # out-copy engine cycle DVE/ACT
        "cp_w": (0, 1),
        # v-copy engine cycle DVE/ACT (vh, v16 per r; vl is always DVE)
        "vc_w": (0, 1),
        "ex_bufs": 8,
        "sc_bufs": 3,       # [128, 2, SQC] head-pair tiles (2 banks each)
        "swpipe": 3,
        "pf_late": True,
        "qadd_pool": True,   # rope adds on the idle Pool engine
        "kadd_pool": True,
        "sbuf_bufs": 3,
        "out_defer": 0,
        "oo_bufs": 6,
        "op_attn": False,
        "rb_bufs": 3,
    }
    if opts:
        o.update(opts)
    nc = bass.Bass()

    xh_d = nc.dram_tensor("xh", (D, S), F8E4, kind="ExternalInput")
    xl_d = nc.dram_tensor("xl", (D, S), F8E5, kind="ExternalInput")
    w_d = {}
    for nm in ("wqh", "wkh", "wvh"):
        w_d[nm] = nc.dram_tensor(nm, (128, KT * 128), F8E4, kind="ExternalInput")
    for nm in ("wql", "wkl", "wvl"):
        w_d[nm] = nc.dram_tensor(nm, (128, KT * 128), F8E5, kind="ExternalInput")
    wo_d = nc.dram_tensor("wo", (128, D), F16, kind="ExternalInput")
    cossin_d = nc.dram_tensor("cossin", (128, 2 * S), F16, kind="ExternalInput")
    if causal:
        gd_d = nc.dram_tensor("gdiag", (128, 4 * SQC), F32, kind="ExternalInput")
    elif mode == "general":
        mask_d = nc.dram_tensor("maskT", (NJ, S, SQC), F32, kind="ExternalInput")
    out_d = nc.dram_tensor("opT", (D, S), BF16, kind="ExternalOutput")

    # which attnV form each mode uses
    use_f8 = mode in ("causal", "zeros")   # e4m3 DR for non-diag tiles
    use_f16v = mode in ("causal", "general")  # f16 v copy for diag/general

    # global engine round-robin state
    cnt = {"exp": 0, "cp": 0, "vc": 0}

    def pick(kind, weights, engines):
        tot = sum(weights)
        i = cnt[kind] % tot
        cnt[kind] += 1
        acc = 0
        for w, e in zip(weights, engines):
            acc += w
            if i < acc:
                return e
        return engines[-1]

    with tile.TileContext(nc) as tc:
        with (
            tc.tile_pool(name="wts", bufs=1) as wts,
            tc.tile_pool(name="big", bufs=1) as big,
            tc.tile_pool(name="xc", bufs=3) as xcp,
            tc.tile_pool(name="cs", bufs=3) as csp,
            tc.tile_pool(name="qs", bufs=o["sbuf_bufs"]) as qsp,
            tc.tile_pool(name="rt", bufs=o["sbuf_bufs"]) as rtp,
            tc.tile_pool(name="qr", bufs=o["sbuf_bufs"]) as qrp,
            tc.tile_pool(name="ex", bufs=o["ex_bufs"]) as exp_p,
            tc.tile_pool(name="mk", bufs=3) as mkp,
            tc.tile_pool(name="af", bufs=o["sbuf_bufs"]) as afp,
            tc.tile_pool(name="rc", bufs=o["sbuf_bufs"]) as rcp,
            tc.tile_pool(name="rb", bufs=o["rb_bufs"]) as rbp,
            tc.tile_pool(name="oo", bufs=o["oo_bufs"]) as oop,
            tc.tile_pool(name="sc", bufs=o["sc_bufs"], space=bass.MemorySpace.PSUM) as scp_p,  # [128,2,SQC] pairs
            tc.tile_pool(name="at", bufs=1, space=bass.MemorySpace.PSUM) as atp,
        ):
            # ---- q weights + chunk-0 inputs first so q-proj(0) starts ASAP
            w_sb = {}
            for nm in ("wqh", "wql", "wkh", "wvh", "wkl", "wvl"):
                dt8 = F8E4 if nm.endswith("h") else F8E5
                w_sb[nm] = wts.tile([128, KT, 128], dt8, tag=nm, name=nm)
            xc0 = xcp.tile([128, KT, SQC], F8E4, tag="xc", name="xc")
            # interleave wqh/xc0 per k-pair: q matmul k starts after ~130KB
            wr_q = w_d["wqh"].rearrange("p (k m) -> p k m", k=KT)
            for k0 in range(0, KT, 2):
                nc.sync.dma_start(
                    w_sb["wqh"][:, k0 : k0 + 2, :], wr_q[:, k0 : k0 + 2, :]
                )
                nc.sync.dma_start(
                    xc0[:, k0 : k0 + 2, :],
                    xh_d[k0 * 128 : (k0 + 2) * 128, 0:SQC].rearrange(
                        "(k p) n -> p k n", p=128
                    ),
                )
            nc.sync.dma_start(
                w_sb["wql"][:], w_d["wql"].rearrange("p (k m) -> p k m", k=KT)
            )
            xcl0 = xcp.tile([128, KT, SQC], F8E5, tag="xcl", name="xcl")
            nc.sync.dma_start(
                xcl0[:], xl_d[:, 0:SQC].rearrange("(k p) n -> p k n", p=128)
            )
            csl0 = csp.tile([128, 2, SQC], F16, tag="cs", name="cs")
            nc.sync.dma_start(
                csl0[:], cossin_d.rearrange("p (c s) -> p c s", c=2)[:, :, 0:SQC]
            )
            # remaining weights in use-order: v before k
            for nm in ("wvh", "wvl", "wkh", "wkl"):
                nc.sync.dma_start(
                    w_sb[nm][:], w_d[nm].rearrange("p (k m) -> p k m", k=KT)
                )
            wo_sb = wts.tile([128, D], F16, tag="wo", name="wo_sb")
            nc.sync.dma_start(wo_sb[:], wo_d[:])
            gd_sb = None
            if causal:
                gd_sb = wts.tile([128, 4, SQC], F32, tag="gd", name="gd_sb")
                nc.sync.dma_start(gd_sb[:], gd_d.rearrange("p (r n) -> p r n", r=4))

            biast8 = wts.tile([128, 1], F32, tag="biast8", name="biast8")
            nc.vector.memset(biast8[:], ACT_BIAS8)

            krot = big.tile([128, S], F16, tag="krot", name="krot")
            qrot_all = big.tile([128, S], F16, tag="qrot", name="qrot_all")
            # fp8 v pairs: per (sk tile, head): [vh(80) | vl(80)] per head,
            # each [dims(64), one, pad] — the pair step (80 B) is 16-aligned
            vext8 = None
            if use_f8:
                vext8 = big.tile([128, NKT, 2, 2, 80], F8E4, tag="v8", name="vext8")
                nc.vector.memset(vext8[:, :, :, 0, 64], 1.0)
                nc.vector.memset(vext8[:, :, :, 1, 64], 0.0)
            # f16 v for diag (pre-scaled by SC8) / general (scale 1)
            vext16 = None
            v16s = (SC8 if causal else 1.0) / 32.0
            if use_f16v:
                vext16 = big.tile([128, NKT, 2, 66], F16, tag="v16", name="vext16")
                nc.vector.memset(vext16[:, :, :, 64], 32.0 * v16s)

            def proj(wname, xc, xcl, ps):
                # hi/lo fp8 DoubleRow: (wh+wl)(xh+xl) ~ wh.xh + wl.xh + wh.xl
                wh = w_sb[wname + "h"]
                wl = w_sb[wname + "l"]
                nmm = 0
                for grp_l, grp_r in ((wh, xc), (wl, xc), (wh, xcl)):
                    for k in range(0, KT, 2):
                        nmm += 1
                        nc.tensor.matmul(
                            ps[:], grp_l[:, k : k + 2, :], grp_r[:, k : k + 2, :],
                            start=(nmm == 1), stop=(nmm == 12),
                            perf_mode=DRMODE,
                        )

            def rot_path(ps, csl, dst_ap, tag):
                # dst = ps*cos + shift(ps*sin'') where sin''[p] = sin'[p^1]
                # (host-prepped): multiplying BEFORE the partition-shift DMA
                # removes the staging copy from the critical chain. Both muls
                # in ONE op (ps broadcast over the cos/sin dim).
                t12 = qsp.tile([128, 2, SQC], F16, tag=f"{tag}16", name=f"{tag}t12")
                nc.vector.tensor_tensor(
                    t12[:],
                    ps[:].unsqueeze(1).to_broadcast((128, 2, SQC)),
                    csl[:, 0:2, :],
                    op=AluOpType.mult,
                )
                t2 = t12[:, 1, :]
                tsh = qsp.tile([128, SQC], F16, tag=f"{tag}sh", name=f"{tag}sh")
                nc.sync.dma_start(tsh[0:127:2, :], t2[1:128:2, :])
                nc.sync.dma_start(tsh[1:128:2, :], t2[0:127:2, :])
                t1 = t12[:, 0, :]
                # the final add is SBUF-only: it can run on the idle Pool
                # engine (k's add is off the critical path; q's gates scores)
                pool_add = o["qadd_pool"] if tag == "q" else o["kadd_pool"]
                if pool_add:
                    nc.gpsimd.tensor_add(dst_ap, t1, tsh[:])
                else:
                    nc.vector.tensor_add(dst_ap, t1, tsh[:])
                return t1, tsh

            def load_chunk(j):
                sl = slice(j * SQC, (j + 1) * SQC)
                # halves: consecutive queued DMAs pipeline at ~2x the rate
                xc = xcp.tile([128, KT, SQC], F8E4, tag="xc", name="xc")
                for k0 in range(0, KT, 4):
                    nc.sync.dma_start(
                        xc[:, k0 : k0 + 4, :],
                        xh_d[k0 * 128 : (k0 + 4) * 128, sl].rearrange(
                            "(k p) n -> p k n", p=128
                        ),
                    )
                xcl = xcp.tile([128, KT, SQC], F8E5, tag="xcl", name="xcl")
                for k0 in range(0, KT, 4):
                    nc.sync.dma_start(
                        xcl[:, k0 : k0 + 4, :],
                        xl_d[k0 * 128 : (k0 + 4) * 128, sl].rearrange(
                            "(k p) n -> p k n", p=128
                        ),
                    )
                csl = csp.tile([128, 2, SQC], F16, tag="cs", name="cs")
                nc.sync.dma_start(
                    csl[:], cossin_d.rearrange("p (c s) -> p c s", c=2)[:, :, sl]
                )
                return xc, xcl, csl

            def emit_vcopies(j, ps_v, r):
                """evacuate one [128,128] v psum tile (2 heads x 64 dims)
                into the fp8 pair planes and/or the f16 diag copy."""
                i = 4 * j + r
                rsl = slice(r * 128, (r + 1) * 128)
                engs = (nc.vector, nc.scalar)
                psv2 = ps_v[:, rsl].rearrange("p (h m) -> p h m", h=2)
                if use_f8:
                    # vh = ps/32 -> e4m3, both heads in one op
                    eng = pick("vc", o["vc_w"], engs)
                    if eng is nc.scalar:
                        nc.scalar.activation(
                            vext8[:, i, :, 0, 0:64], psv2,
                            AF.Copy, bias=0.0, scale=1.0 / 32.0,
                        )
                    else:
                        nc.vector.tensor_scalar_mul(
                            vext8[:, i, :, 0, 0:64], psv2, 1.0 / 32.0
                        )
                    # vl = ps/32 - vh, ALSO e4m3 (the DR pair AP has a single
                    # dtype); e4m3 residual keeps v at ~bf16 precision
                    nc.vector.scalar_tensor_tensor(
                        vext8[:, i, :, 1, 0:64],
                        psv2,
                        1.0 / 32.0,
                        vext8[:, i, :, 0, 0:64],
                        AluOpType.mult,
                        AluOpType.subtract,
                    )
                if use_f16v:
                    eng = pick("vc", o["vc_w"], engs)
                    if eng is nc.scalar:
                        nc.scalar.activation(
                            vext16[:, i, :, 0:64], psv2,
                            AF.Copy, bias=0.0, scale=v16s,
                        )
                    else:
                        nc.vector.tensor_scalar_mul(
                            vext16[:, i, :, 0:64], psv2, v16s
                        )

            def do_proj_chunk(j, loaded, nxt):
                sl = slice(j * SQC, (j + 1) * SQC)
                xc, xcl, csl = loaded
                ps_q = scp_p.tile([128, 2, SQC], F32, tag="scp", name="ps_q")[:, 0, :]
                proj("wq", xc, xcl, ps_q)
                rot_path(ps_q, csl, qrot_all[:, sl], "q")
                nxt_loaded = None
                if not o["pf_late"]:
                    nxt_loaded = load_chunk(nxt) if nxt is not None else None
                # v (transposed): out [sk, hd] per 128-sk tile
                ps_v = scp_p.tile([128, 2, SQC], F32, tag="scp", name="ps_v")[:, 0, :]
                for r in range(4):
                    rsl = slice(r * 128, (r + 1) * 128)
                    nmm = 0
                    for grp_l, grp_r in (
                        (xc, w_sb["wvh"]), (xc, w_sb["wvl"]), (xcl, w_sb["wvh"])
                    ):
                        for k in range(0, KT, 2):
                            nmm += 1
                            nc.tensor.matmul(
                                ps_v[:, rsl],
                                grp_l[:, k : k + 2, rsl],
                                grp_r[:, k : k + 2, :],
                                start=(nmm == 1), stop=(nmm == 12),
                                skip_group_check=True,
                                perf_mode=DRMODE,
                            )
                    emit_vcopies(j, ps_v, r)
                ps_k = scp_p.tile([128, 2, SQC], F32, tag="scp", name="ps_k")[:, 0, :]
                proj("wk", xc, xcl, ps_k)
                rot_path(ps_k, csl, krot[:, sl], "k")
                # prefetch next chunk's inputs AFTER the latency-critical
                # q/k shift DMAs so they are not stuck behind a big transfer
                if o["pf_late"]:
                    nxt_loaded = load_chunk(nxt) if nxt is not None else None
                return nxt_loaded

            def emit_exp(scp2, i, j, qlo):
                """one op over BOTH heads' [128, 2, SQC-qlo] scores.
                returns (ex2_tile, kind): kind 'f8' or 'f16'"""
                diag = causal and i >= 4 * j
                if causal or mode == "zeros":
                    if diag:
                        ex = exp_p.tile([128, 2, SQC], F16, tag="ex16", name="ex16")
                        g = (
                            gd_sb[:, i - 4 * j, qlo:]
                            .unsqueeze(1)
                            .to_broadcast((128, 2, SQC - qlo))
                        )
                        nc.vector.scalar_tensor_tensor(
                            ex[:, :, qlo:].bitcast(I16),
                            scp2[:, :, qlo:], EBIAS, g,
                            AluOpType.add, AluOpType.min,
                        )
                        return ex, "f16"
                    ex = exp_p.tile([128, 2, SQC], F8E4, tag="ex8", name="ex8")
                    eng = pick("exp", o["exp_w"], (nc.scalar, nc.vector))
                    if eng is nc.scalar:
                        nc.scalar.activation(
                            ex[:], scp2[:], AF.Exp, bias=biast8[:], scale=ACT_SCALE
                        )
                    else:
                        nc.vector.tensor_scalar(
                            ex[:].bitcast(U8), scp2[:], 1.0 / 128.0, BIAS8,
                            AluOpType.mult, AluOpType.add,
                        )
                    return ex, "f8"
                # general: additive mask (same rows for both heads), fp16 trick
                mt = mkp.tile([128, SQC], F32, tag="mk", name="mt")
                nc.sync.dma_start(mt[:], mask_d[j, i * 128 : (i + 1) * 128, :])
                t = mkp.tile([128, 2, SQC], F32, tag="mks", name="mts")
                nc.vector.tensor_tensor(
                    t[:], scp2[:],
                    mt[:].unsqueeze(1).to_broadcast((128, 2, SQC)),
                    op=AluOpType.add,
                )
                ex = exp_p.tile([128, 2, SQC], F16, tag="ex16", name="ex16")
                t2 = mkp.tile([128, 2, SQC], F32, tag="mks2", name="mts2")
                nc.vector.tensor_scalar(
                    t2[:], t[:], EBIAS, ECLAMP, AluOpType.add, AluOpType.min
                )
                nc.vector.tensor_scalar_max(ex[:].bitcast(I16), t2[:], 0.0)
                return ex, "f16"

            def do_attn_chunk(j, oproj_after=None):
                sl = slice(j * SQC, (j + 1) * SQC)
                nkt_j = 4 * (j + 1) if causal else NKT
                at2 = atp.tile([128, 2, SQC], F32, tag="at", name="at2")
                pend = []

                def emit_scores(i):
                    # diagonal tiles only have valid queries n >= (i-4j)*128
                    qlo = (i - 4 * j) * 128 if (causal and i > 4 * j) else 0
                    scp2 = scp_p.tile([128, 2, SQC], F32, tag="scp", name="scp")
                    for h in range(HPC):
                        hsl = slice(h * 64, (h + 1) * 64)
                        nc.tensor.matmul(
                            scp2[:, h, qlo:],
                            krot[hsl, i * 128 : (i + 1) * 128],
                            qrot_all[hsl, j * SQC + qlo : (j + 1) * SQC],
                            start=True, stop=True,
                            skip_group_check=True,
                        )
                    ex, kind = emit_exp(scp2, i, j, qlo)
                    return ex, qlo, kind

                def emit_attnv(i, ex2, qlo, kind, first, last):
                    for h in range(HPC):
                        out_ap = at2[0:65, h, qlo:]
                        if kind == "f8":
                            rhs = (
                                ex2[:, h, :]
                                .unsqueeze(1)
                                .to_broadcast((128, 2, SQC))
                            )
                            nc.tensor.matmul(
                                out_ap,
                                vext8[:, i, h, :, 0:65],
                                rhs,
                                start=first, stop=last,
                                perf_mode=DRMODE,
                                skip_group_check=True,
                            )
                        else:
                            nc.tensor.matmul(
                                out_ap,
                                vext16[:, i, h, 0:65],
                                ex2[:, h, qlo:],
                                start=first, stop=last,
                                skip_group_check=True,
                            )

                # tile order: spread the diagonal tiles (whose exp is
                # DVE-only) evenly among non-diag tiles so the two exp
                # engines stay co-busy through the whole chunk
                if causal:
                    m = 4 * j
                    dslots = {((k + 1) * (m + 4)) // 5: k for k in range(4)}
                    order = []
                    ni = di = 0
                    for s in range(m + 4):
                        if (s in dslots and di <= dslots[s]) or ni >= m:
                            order.append(m + di)
                            di += 1
                        else:
                            order.append(ni)
                            ni += 1
                    while di < 4:
                        order.append(m + di)
                        di += 1
                    assert sorted(order) == list(range(nkt_j)), order
                else:
                    order = list(range(nkt_j))

                depth = o["swpipe"]
                hooks = list(oproj_after or [])
                op_emitted = 0
                op_total = len(hooks) * (KT // 2)
                for n_e, i in enumerate(order):
                    ex2, qlo, kind = emit_scores(i)
                    pend.append((i, ex2, qlo, kind, n_e == 0, n_e == nkt_j - 1))
                    # deferred o-proj pairs land spread through this chunk's
                    # score stream (afin long ready; each pair claims a
                    # score-ring slot and its copy finishes before the slot
                    # is needed again)
                    if op_emitted < op_total and n_e % 2 == 1:
                        hooks[op_emitted // (KT // 2)][0](op_emitted % (KT // 2))
                        op_emitted += 1
                    if len(pend) > depth:
                        emit_attnv(*pend.pop(0))
                while op_emitted < op_total:
                    hooks[op_emitted // (KT // 2)][0](op_emitted % (KT // 2))
                    op_emitted += 1
                for hk in hooks:
                    hk[1]()
                for p in pend:
                    emit_attnv(*p)

                # ONE reciprocal over both heads' denominator rows, then DMA
                # partition-broadcasts (stride-0 free-dim read replicates the
                # one-partition row across 64 partitions).
                rec = rcp.tile([65, 2, SQC], F32, tag="rec", name="rec")
                with nc.allow_low_precision("reciprocal of softmax denom"):
                    nc.vector.reciprocal(rec[64:65, :, :], at2[64:65, :, :])
                recbs = []
                for h in range(HPC):
                    recb = rbp.tile([64, SQC], F32, tag=f"recb{h}", name=f"recb{h}")
                    nc.sync.dma_start(
                        recb[:],
                        rec[64:65, h, :].unsqueeze(1).to_broadcast((1, 64, SQC)),
                    )
                    recbs.append(recb)

                def norm_part():
                    # normalize directly from attnV PSUM against the DMA-
                    # broadcast reciprocal rows; h1 first so its partition-
                    # move DMA overlaps h0's work
                    afin = afp.tile([128, SQC], F16, tag="afin", name="afin")
                    tmph = afp.tile([64, SQC], F16, tag="tmph", name="tmph")
                    nc.vector.tensor_tensor(
                        tmph[:], at2[0:64, 1, :], recbs[1][:],
                        op=AluOpType.mult,
                    )
                    nc.sync.dma_start(afin[64:128, :], tmph[:])
                    nc.vector.tensor_tensor(
                        afin[0:64, :], at2[0:64, 0, :], recbs[0][:],
                        op=AluOpType.mult,
                    )
                    return afin

                def oproj_pairs(afin):
                    """generator of per-pair emitters: the o-proj matmuls
                    write into freed at-pool pair tiles, spread through the
                    next chunk's score stream so copies finish between."""
                    os_big = oop.tile([128, KT, SQC], BF16, tag="oo", name="os_big")

                    def one(dp, drain=False):
                        op2 = scp_p.tile([128, 2, SQC], F32, tag="scp", name="op2")
                        for g in range(2):
                            dt_i = 2 * dp + g
                            nc.tensor.matmul(
                                op2[:, g, :],
                                wo_sb[:, dt_i * 128 : (dt_i + 1) * 128],
                                afin[:],
                                start=True, stop=True,
                                skip_group_check=True,
                            )
                        if drain:
                            eng = nc.vector if dp % 2 == 0 else nc.scalar
                        else:
                            eng = pick("cp", o["cp_w"], (nc.vector, nc.scalar))
                        if eng is nc.scalar:
                            nc.scalar.copy(
                                os_big[:, 2 * dp : 2 * dp + 2, :], op2[:]
                            )
                        else:
                            eng.tensor_copy(
                                os_big[:, 2 * dp : 2 * dp + 2, :], op2[:]
                            )

                    def flush(drain=False):
                        if drain:
                            for dt_i in range(KT):
                                nc.sync.dma_start(
                                    out_d[dt_i * 128 : (dt_i + 1) * 128, sl],
                                    os_big[:, dt_i, :],
                                )
                        else:
                            nc.sync.dma_start(
                                out_d[:, sl].rearrange("(k p) n -> p k n", p=128),
                                os_big[:],
                            )

                    return one, flush

                return (norm_part, oproj_pairs)

            # Projections run one chunk AHEAD of attention: proj(j) then
            # attn(j-1). The x-load DMAs stream behind attention compute,
            # and attention's exp-engine load overlaps projection's PE burst.
            # Chunk a's normalize runs at the start of attn(a+1); its o-proj
            # pairs spread through attn(a+2)'s score stream.
            pend_out = []
            state = {"pend_norm": None, "ready_op": None}

            def attn_step(aj, last=False):
                hooks = []
                if state["ready_op"] is not None:
                    hooks.append(state["ready_op"])
                    state["ready_op"] = None
                if state["pend_norm"] is not None:
                    np_, opg_ = state["pend_norm"]
                    afin_prev = np_()
                    state["ready_op"] = opg_(afin_prev)
                    state["pend_norm"] = None
                if last and state["ready_op"] is not None:
                    hooks.append(state["ready_op"])
                    state["ready_op"] = None
                state["pend_norm"] = do_attn_chunk(aj, oproj_after=hooks or None)

            loaded = (xc0, xcl0, csl0)
            loaded_nxt = load_chunk(1)  # 2-deep x prefetch (xc bufs=3)
            for j in range(NJ):
                nxt = j + 2 if j + 2 < NJ else None
                new_loaded = do_proj_chunk(j, loaded, nxt)
                if j >= 1:
                    attn_step(j - 1)
                loaded = loaded_nxt
                loaded_nxt = new_loaded
            attn_step(NJ - 1, last=True)
            # tail: the last chunk's normalize + o-proj drain
            np_, opg_ = state["pend_norm"]
            afin_last = np_()
            one, flush = opg_(afin_last)
            for dp in range(KT // 2):
                one(dp, drain=True)
            flush(drain=True)

    _fix_waits(nc)
    return nc


def _host_prep(x, cos, sin, mask, wq, wk, wv, wo):
    x = np.asarray(x, dtype=np.float32)
    cos = np.asarray(cos, dtype=np.float32)
    sin = np.asarray(sin, dtype=np.float32)
    mask = np.asarray(mask, dtype=np.float32)
    wq = np.asarray(wq, dtype=np.float32)
    wk = np.asarray(wk, dtype=np.float32)
    wv = np.asarray(wv, dtype=np.float32)
    wo = np.asarray(wo, dtype=np.float32)
    import ml_dtypes

    E4 = ml_dtypes.float8_e4m3
    E5 = ml_dtypes.float8_e5m2
    xT = np.ascontiguousarray(x.reshape(S, D).T)
    xh = xT.astype(E4)
    xl = (xT - xh.astype(np.float32)).astype(E5)

    # cos/sin rows: partition p -> rotation pair (p % 64)//2; sign on sin
    idx = np.repeat(np.arange(HD // 2), 2)                 # (64,)
    cosr = cos[:, idx].T                                   # (64, S)
    sinr = sin[:, idx].T
    sgn = np.where(np.arange(HD) % 2 == 0, -1.0, 1.0)[:, None]
    csc = CS / 32.0                       # undo the 32x fp8 weight rescale
    cos128 = np.vstack([cosr, cosr]) * csc
    sin128 = np.vstack([sinr * sgn, sinr * sgn]) * csc
    # sin''[p] = sin'[p^1]: the mul happens BEFORE the partition shift
    sin128 = sin128.reshape(64, 2, S)[:, ::-1, :].reshape(128, S)
    cossin = np.concatenate(
        [cos128[:, None, :], sin128[:, None, :]], axis=1
    ).reshape(128, 2 * S).astype(np.float16)

    neg = np.isneginf(mask)
    triu = np.triu(np.ones((S, S), dtype=bool), 1)
    if not neg.any() and not mask.any():
        mode = "zeros"
    elif np.array_equal(neg, triu) and not mask[~neg].any():
        mode = "causal"
        blk0 = mask[0:SQC, 0:SQC]
        # gate[p, r, n] over score tile [sk=128, sq=512]: valid iff not -inf
        gd = np.empty((128, 4, SQC), np.float32)
        for r in range(4):
            blkv = ~np.isneginf(blk0[:, r * 128 : (r + 1) * 128])  # (q, k)
            gd[:, r, :] = np.where(blkv.T, ECLAMP, 0.0)
        gdiag = np.ascontiguousarray(gd.reshape(128, 4 * SQC))
    else:
        mode = "general"
        maskT = np.empty((NJ, S, SQC), np.float32)
        for j in range(NJ):
            maskT[j] = mask[j * SQC : (j + 1) * SQC, :].T * np.float32(L2E1024)

    per_core = []
    for c in range(NC):
        hs, he = c * 128, (c + 1) * 128
        m = {"xh": xh, "xl": xl, "cossin": cossin}
        # lhsT weight layout: w_t[p, k*128+m] = 32 * w_slice[m, k*128+p]
        for name, w in (("wq", wq), ("wk", wk), ("wv", wv)):
            ws = w[hs:he, :] * np.float32(32.0)            # (128, D)
            t = ws.T.reshape(KT, 128, 128)                 # (k, p, m)
            wt = np.ascontiguousarray(t.transpose(1, 0, 2).reshape(128, D))
            whq = wt.astype(E4)
            m[name + "h"] = whq
            m[name + "l"] = (wt - whq.astype(np.float32)).astype(E5)
        m["wo"] = np.ascontiguousarray(wo[:, hs:he].T).astype(np.float16)
        if mode == "causal":
            m["gdiag"] = gdiag
        elif mode == "general":
            m["maskT"] = maskT
        per_core.append(m)
    return mode, per_core


_cache = {}


def kernel(x, cos, sin, mask, wq, wk, wv, wo, start_pos=0, **_):
    mode, in_maps = _host_prep(x, cos, sin, mask, wq, wk, wv, wo)
    if mode not in _cache:
        _cache[mode] = build_program(mode)
    nc = _cache[mode]
    res = run_bass_kernel_spmd(nc, in_maps, core_ids=list(range(NC)))
    acc = np.zeros((D, S), np.float64)
    for c in range(NC):
        acc += res.results[c]["opT"].astype(np.float64)
    return np.ascontiguousarray(acc.T).reshape(B, S, D).astype(np.float32)


# revision 71
# speedup vs baseline: 1.0636x; 1.0636x over previous
"""Multi-head causal attention with RoPE (B=1, S=4096, D=1024, H=16) on 8
Trainium2 NeuronCores.

Sharding: tensor-parallel over heads - each core computes 2 heads (QKV
projections column-sliced, attention, and its rank-128 partial of the output
projection; host sums the 8 partials = row-parallel wo).

Design (v4):
  - QKV projections in hi/lo fp8 DoubleRow form: x = xh(e4m3)+xl(e5m2) and
    w*32 = wh(e4m3)+wl(e5m2) host-side; (wh.xh + wl.xh + wh.xl) via three
    DoubleRow groups (K=256/instr, 0.5 cyc/row); v is computed already
    transposed ([seq, head_dim]) by swapping matmul operands.
  - RoPE without swap-projections: DVE muls by host-prepped cos/sin rows, a
    partition-shift SBUF DMA builds the pair-partner tensor, and the final
    SBUF-only add runs on the otherwise-idle Pool (GpSimd) engine.
  - scores in fp16 at 1 cyc/row; above-diagonal 128x512 tiles are skipped
    and diagonal tiles narrowed to their valid query range.
  - attnV in fp8 DoubleRow at 0.5 cyc/row: lhsT pairs (v_hi e4m3, v_lo e5m2)
    against a stride-0-duplicated fp8 ex rhs - v at ~bf16 precision, half
    the PE cost. ex for non-diagonal tiles is e4m3: on DVE one
    tensor_scalar (y*2^-7 + 20) with round+saturate into uint8 bits
    (negatives saturate to 0 = e4m3 +0.0); on ACT a native exp with output
    cast to f8e4 (bias matches the bit-trick's mean ratio). Diagonal tiles
    keep the fp16 Schraudolph trick with the 0/31743 mask-folding gate and
    multiply a separate f16 v copy pre-scaled by 2^-4.5 so both paths land
    on the same absolute scale (e4 trick value = 2^((bits-56)/8)).
  - denominators from a ones column in the v tiles; reciprocal rows are
    partition-broadcast by DMA (free-dim stride-0 read) and the normalize
    muls read the attnV PSUM directly; the h1 accumulator lives at
    partitions 63:128 (ones row first) so afin[64:128] is written in place
    with no partition-move DMA; o-proj is a single K=128 fp16 matmul per
    128-dout tile.
  - scheduling: per-chunk epilogue (normalize+o-proj) is deferred past the
    NEXT chunk's projections; exp ops and PSUM-evacuation copies are
    round-robined over ACT and DVE; next-chunk x/cos DMAs prefetch behind
    the latency-critical shift DMAs; the last chunk writes its output in
    per-tile DMAs to shorten the drain.
"""
import math
import numpy as np

import concourse.bass as bass
import concourse.mybir as mybir
import concourse.tile as tile
from concourse.bass_utils import run_bass_kernel_spmd
from concourse.alu_op_type import AluOpType

B, S, D, H = 1, 4096, 1024, 16
HD = D // H            # 64
NC = 8                 # cores
HPC = H // NC          # 2 heads per core
SQC = 512              # seq chunk (matmul free dim)
NJ = S // SQC          # 8 chunks
NKT = S // 128         # 32 sk partition tiles
KT = D // 128          # 8 contraction tiles for projections

F32 = mybir.dt.float32
F16 = mybir.dt.float16
BF16 = mybir.dt.bfloat16
I16 = mybir.dt.int16
U8 = mybir.dt.uint8
F8E4 = mybir.dt.float8e4
F8E5 = mybir.dt.float8e5
DRMODE = mybir.MatmulPerfMode.DoubleRow
AF = mybir.ActivationFunctionType

L2E1024 = math.log2(math.e) * 1024.0          # 1477.3197 (y = z * this)
CS = math.sqrt(L2E1024 / 8.0)                 # 13.5891 folded into cos/sin
EBIAS = 15360.0                               # fp16 exponent bias << 10
ECLAMP = 31743.0                              # just below fp16 +inf bits
ACT_SCALE = 1.0 / L2E1024
RMEAN = 1.0406936                             # bit-trick mean ratio
# e4m3 trick: bits = z*8*log2e + BIAS8 = y/128 + BIAS8 (uint8, sat at 0)
BIAS8 = 20.0
SC8 = 2.0 ** ((BIAS8 - 56.0) / 8.0)           # e4 trick global scale 2^-4.5
ACT_BIAS8 = math.log(RMEAN * SC8)             # ACT exp out matches trick
# diag f16 path value = r*e^z (scale 1); bridge via v16 pre-scale SC8

_MAX_WAITS = 1


def _fix_waits(nc):
    """walrus in this container rejects >1 sync-wait per instruction
    ("Too many sync wait commands"); split excess waits onto preceding
    same-engine NoOps (engine blocks in order, semantics preserved)."""
    n = 0
    for fn in nc.m.functions:
        for bb in fn.blocks:
            new_list = []
            for inst in bb.instructions:
                si = getattr(inst, "sync_info", None)
                if si is not None and si.on_wait and len(si.on_wait) > _MAX_WAITS:
                    waits = list(si.on_wait)
                    excess, keep = waits[:-_MAX_WAITS], waits[-_MAX_WAITS:]
                    for j in range(0, len(excess), _MAX_WAITS):
                        nop = mybir.InstNoOp(
                            name=f"I-waitfix-{nc.next_id()}",
                            ins=[],
                            outs=[],
                            engine=inst.engine,
                            sync_info=mybir.SyncInfo(
                                on_wait=excess[j : j + _MAX_WAITS], on_update=[]
                            ),
                        )
                        nc.register_instruction(nop)
                        new_list.append(nop)
                        n += 1
                    si.on_wait = keep
                new_list.append(inst)
            bb.instructions[:] = new_list
    return n


def build_program(mode: str, opts=None):
    """mode: 'causal' (skip above-diag tiles; gate tiles fold the mask),
    'zeros' (no mask, full attention), 'general' (additive mask, full)."""
    causal = mode == "causal"
    o = {
        # non-diag exp engine shares: ACT (f8 native exp) / DVE (u8 trick);
        # late chunks saturate ACT first -> shift their split toward DVE
        "exp_w": (2, 1),
        "exp_w2": (1, 1),
        "exp_late": 4,
        # out-copy engine cycle DVE/ACT
        "cp_w": (0, 1),
        # v-copy engine cycle DVE/ACT (vh, v16 per r; vl is always DVE)
        "vc_w": (0, 1),
        "ex_bufs": 10,
        "sc_bufs": 3,       # [128, 2, SQC] head-pair tiles (2 banks each)
        "swpipe": 4,
        "pf_late": True,
        "qadd_pool": True,   # rope adds on the idle Pool engine
        "kadd_pool": True,
        "fine_ilv": False,   # spread proj sub-blocks through the attn stream
        "bl_every": 2,
        "sbuf_bufs": 3,
        "out_defer": 0,
        "oo_bufs": 6,
        "op_attn": False,
        "rb_bufs": 3,
    }
    if opts:
        o.update(opts)
    nc = bass.Bass()

    xh_d = nc.dram_tensor("xh", (D, S), F8E4, kind="ExternalInput")
    xl_d = nc.dram_tensor("xl", (D, S), F8E5, kind="ExternalInput")
    w_d = {}
    for nm in ("wqh", "wkh", "wvh"):
        w_d[nm] = nc.dram_tensor(nm, (128, KT * 128), F8E4, kind="ExternalInput")
    for nm in ("wql", "wkl", "wvl"):
        w_d[nm] = nc.dram_tensor(nm, (128, KT * 128), F8E5, kind="ExternalInput")
    wo_d = nc.dram_tensor("wo", (128, D), F16, kind="ExternalInput")
    cossin_d = nc.dram_tensor("cossin", (128, 2 * S), F16, kind="ExternalInput")
    if causal:
        gd_d = nc.dram_tensor("gdiag", (128, 4 * SQC), F16, kind="ExternalInput")
    elif mode == "general":
        mask_d = nc.dram_tensor("maskT", (NJ, S, SQC), F32, kind="ExternalInput")
    out_d = nc.dram_tensor("opT", (D, S), BF16, kind="ExternalOutput")

    # which attnV form each mode uses
    use_f8 = mode in ("causal", "zeros")   # e4m3 DR for non-diag tiles
    use_f16v = mode in ("causal", "general")  # f16 v copy for diag/general

    # global engine round-robin state
    cnt = {"exp": 0, "cp": 0, "vc": 0}

    def pick(kind, weights, engines):
        tot = sum(weights)
        i = cnt[kind] % tot
        cnt[kind] += 1
        acc = 0
        for w, e in zip(weights, engines):
            acc += w
            if i < acc:
                return e
        return engines[-1]

    with tile.TileContext(nc) as tc:
        with (
            tc.tile_pool(name="wts", bufs=1) as wts,
            tc.tile_pool(name="big", bufs=1) as big,
            tc.tile_pool(name="xc", bufs=3) as xcp,
            tc.tile_pool(name="cs", bufs=3) as csp,
            tc.tile_pool(name="qs", bufs=o["sbuf_bufs"]) as qsp,
            tc.tile_pool(name="rt", bufs=o["sbuf_bufs"]) as rtp,
            tc.tile_pool(name="qr", bufs=o["sbuf_bufs"]) as qrp,
            tc.tile_pool(name="ex", bufs=o["ex_bufs"]) as exp_p,
            tc.tile_pool(name="mk", bufs=3) as mkp,
            tc.tile_pool(name="af", bufs=o["sbuf_bufs"]) as afp,
            tc.tile_pool(name="rc", bufs=o["sbuf_bufs"]) as rcp,
            tc.tile_pool(name="rb", bufs=o["rb_bufs"]) as rbp,
            tc.tile_pool(name="oo", bufs=o["oo_bufs"]) as oop,
            tc.tile_pool(name="sc", bufs=o["sc_bufs"], space=bass.MemorySpace.PSUM) as scp_p,  # [128,2,SQC] pairs
            tc.tile_pool(name="at", bufs=1, space=bass.MemorySpace.PSUM) as atp,
        ):
            # ---- q weights + chunk-0 inputs first so q-proj(0) starts ASAP
            w_sb = {}
            for nm in ("wqh", "wql", "wkh", "wvh", "wkl", "wvl"):
                dt8 = F8E4 if nm.endswith("h") else F8E5
                w_sb[nm] = wts.tile([128, KT, 128], dt8, tag=nm, name=nm)
            xc0 = xcp.tile([128, KT, SQC], F8E4, tag="xc", name="xc")
            # interleave wqh/xc0 per k-pair: q matmul k starts after ~130KB
            wr_q = w_d["wqh"].rearrange("p (k m) -> p k m", k=KT)
            for k0 in range(0, KT, 2):
                nc.sync.dma_start(
                    w_sb["wqh"][:, k0 : k0 + 2, :], wr_q[:, k0 : k0 + 2, :]
                )
                nc.sync.dma_start(
                    xc0[:, k0 : k0 + 2, :],
                    xh_d[k0 * 128 : (k0 + 2) * 128, 0:SQC].rearrange(
                        "(k p) n -> p k n", p=128
                    ),
                )
            nc.sync.dma_start(
                w_sb["wql"][:], w_d["wql"].rearrange("p (k m) -> p k m", k=KT)
            )
            xcl0 = xcp.tile([128, KT, SQC], F8E5, tag="xcl", name="xcl")
            nc.sync.dma_start(
                xcl0[:], xl_d[:, 0:SQC].rearrange("(k p) n -> p k n", p=128)
            )
            csl0 = csp.tile([128, 2, SQC], F16, tag="cs", name="cs")
            nc.sync.dma_start(
                csl0[:], cossin_d.rearrange("p (c s) -> p c s", c=2)[:, :, 0:SQC]
            )
            # remaining weights in use-order: v before k
            for nm in ("wvh", "wvl", "wkh", "wkl"):
                nc.sync.dma_start(
                    w_sb[nm][:], w_d[nm].rearrange("p (k m) -> p k m", k=KT)
                )
            wo_sb = wts.tile([128, D], F16, tag="wo", name="wo_sb")
            nc.sync.dma_start(wo_sb[:], wo_d[:])
            gd_sb = None
            if causal:
                gd_sb = wts.tile([128, 4, SQC], F16, tag="gd", name="gd_sb")
                nc.sync.dma_start(gd_sb[:], gd_d.rearrange("p (r n) -> p r n", r=4))

            biast8 = wts.tile([128, 1], F32, tag="biast8", name="biast8")
            nc.vector.memset(biast8[:], ACT_BIAS8)

            krot = big.tile([128, S], F16, tag="krot", name="krot")
            qrot_all = big.tile([128, S], F16, tag="qrot", name="qrot_all")
            # fp8 v pairs: per (sk tile, head): [vh(80) | vl(80)] per head,
            # each [dims(64), one, pad] — the pair step (80 B) is 16-aligned
            vext8 = None
            if use_f8:
                vext8 = big.tile([128, NKT, 2, 2, 80], F8E4, tag="v8", name="vext8")
                nc.vector.memset(vext8[:, :, :, 0, 64], 1.0)
                nc.vector.memset(vext8[:, :, :, 1, 64], 0.0)
            # f16 v for diag (pre-scaled by SC8) / general (scale 1)
            vext16 = None
            v16s = (SC8 if causal else 1.0) / 32.0
            if use_f16v:
                vext16 = big.tile([128, NKT, 2, 66], F16, tag="v16", name="vext16")
                nc.vector.memset(vext16[:, :, :, 64], 32.0 * v16s)

            def proj(wname, xc, xcl, ps):
                # hi/lo fp8 DoubleRow: (wh+wl)(xh+xl) ~ wh.xh + wl.xh + wh.xl
                wh = w_sb[wname + "h"]
                wl = w_sb[wname + "l"]
                nmm = 0
                for grp_l, grp_r in ((wh, xc), (wl, xc), (wh, xcl)):
                    for k in range(0, KT, 2):
                        nmm += 1
                        nc.tensor.matmul(
                            ps[:], grp_l[:, k : k + 2, :], grp_r[:, k : k + 2, :],
                            start=(nmm == 1), stop=(nmm == 12),
                            perf_mode=DRMODE,
                        )

            def rot_path(ps, csl, dst_ap, tag):
                # dst = ps*cos + shift(ps*sin'') where sin''[p] = sin'[p^1]
                # (host-prepped): multiplying BEFORE the partition-shift DMA
                # removes the staging copy from the critical chain. Both muls
                # in ONE op (ps broadcast over the cos/sin dim).
                t12 = qsp.tile([128, 2, SQC], F16, tag=f"{tag}16", name=f"{tag}t12")
                nc.vector.tensor_tensor(
                    t12[:],
                    ps[:].unsqueeze(1).to_broadcast((128, 2, SQC)),
                    csl[:, 0:2, :],
                    op=AluOpType.mult,
                )
                t2 = t12[:, 1, :]
                tsh = qsp.tile([128, SQC], F16, tag=f"{tag}sh", name=f"{tag}sh")
                nc.sync.dma_start(tsh[0:127:2, :], t2[1:128:2, :])
                nc.sync.dma_start(tsh[1:128:2, :], t2[0:127:2, :])
                t1 = t12[:, 0, :]
                # the final add is SBUF-only: it can run on the idle Pool
                # engine (k's add is off the critical path; q's gates scores)
                pool_add = o["qadd_pool"] if tag == "q" else o["kadd_pool"]
                if pool_add:
                    nc.gpsimd.tensor_add(dst_ap, t1, tsh[:])
                else:
                    nc.vector.tensor_add(dst_ap, t1, tsh[:])
                return t1, tsh

            def load_chunk(j):
                sl = slice(j * SQC, (j + 1) * SQC)
                # halves: consecutive queued DMAs pipeline at ~2x the rate
                xc = xcp.tile([128, KT, SQC], F8E4, tag="xc", name="xc")
                for k0 in range(0, KT, 4):
                    nc.sync.dma_start(
                        xc[:, k0 : k0 + 4, :],
                        xh_d[k0 * 128 : (k0 + 4) * 128, sl].rearrange(
                            "(k p) n -> p k n", p=128
                        ),
                    )
                xcl = xcp.tile([128, KT, SQC], F8E5, tag="xcl", name="xcl")
                for k0 in range(0, KT, 4):
                    nc.sync.dma_start(
                        xcl[:, k0 : k0 + 4, :],
                        xl_d[k0 * 128 : (k0 + 4) * 128, sl].rearrange(
                            "(k p) n -> p k n", p=128
                        ),
                    )
                csl = csp.tile([128, 2, SQC], F16, tag="cs", name="cs")
                nc.sync.dma_start(
                    csl[:], cossin_d.rearrange("p (c s) -> p c s", c=2)[:, :, sl]
                )
                return xc, xcl, csl

            def emit_vcopies(j, ps_v, r):
                """evacuate one [128,128] v psum tile (2 heads x 64 dims)
                into the fp8 pair planes and/or the f16 diag copy."""
                i = 4 * j + r
                rsl = slice(r * 128, (r + 1) * 128)
                engs = (nc.vector, nc.scalar)
                psv2 = ps_v[:, rsl].rearrange("p (h m) -> p h m", h=2)
                if use_f8:
                    # vh = ps/32 -> e4m3, both heads in one op
                    eng = pick("vc", o["vc_w"], engs)
                    if eng is nc.scalar:
                        nc.scalar.activation(
                            vext8[:, i, :, 0, 0:64], psv2,
                            AF.Copy, bias=0.0, scale=1.0 / 32.0,
                        )
                    else:
                        nc.vector.tensor_scalar_mul(
                            vext8[:, i, :, 0, 0:64], psv2, 1.0 / 32.0
                        )
                    # vl = ps/32 - vh, ALSO e4m3 (the DR pair AP has a single
                    # dtype); e4m3 residual keeps v at ~bf16 precision
                    nc.vector.scalar_tensor_tensor(
                        vext8[:, i, :, 1, 0:64],
                        psv2,
                        1.0 / 32.0,
                        vext8[:, i, :, 0, 0:64],
                        AluOpType.mult,
                        AluOpType.subtract,
                    )
                if use_f16v:
                    eng = pick("vc", o["vc_w"], engs)
                    if eng is nc.scalar:
                        nc.scalar.activation(
                            vext16[:, i, :, 0:64], psv2,
                            AF.Copy, bias=0.0, scale=v16s,
                        )
                    else:
                        nc.vector.tensor_scalar_mul(
                            vext16[:, i, :, 0:64], psv2, v16s
                        )

            def proj_blocks(j, loaded, nxt, out):
                """returns a list of sub-block emitters for chunk j's
                projections, to be spread through an attention tile stream.
                `out` is a 1-slot list receiving the prefetched next chunk."""
                sl = slice(j * SQC, (j + 1) * SQC)
                xc, xcl, csl = loaded

                def b_q():
                    ps_q = scp_p.tile(
                        [128, 2, SQC], F32, tag="scp", name="ps_q"
                    )[:, 0, :]
                    proj("wq", xc, xcl, ps_q)
                    rot_path(ps_q, csl, qrot_all[:, sl], "q")

                def b_v(r0):
                    def f():
                        ps_v = scp_p.tile(
                            [128, 2, SQC], F32, tag="scp", name="ps_v"
                        )[:, 0, :]
                        for r in (r0, r0 + 1):
                            rsl = slice(r * 128, (r + 1) * 128)
                            nmm = 0
                            for grp_l, grp_r in (
                                (xc, w_sb["wvh"]),
                                (xc, w_sb["wvl"]),
                                (xcl, w_sb["wvh"]),
                            ):
                                for k in range(0, KT, 2):
                                    nmm += 1
                                    nc.tensor.matmul(
                                        ps_v[:, rsl],
                                        grp_l[:, k : k + 2, rsl],
                                        grp_r[:, k : k + 2, :],
                                        start=(nmm == 1), stop=(nmm == 12),
                                        skip_group_check=True,
                                        perf_mode=DRMODE,
                                    )
                            emit_vcopies(j, ps_v, r)
                    return f

                def b_k():
                    ps_k = scp_p.tile(
                        [128, 2, SQC], F32, tag="scp", name="ps_k"
                    )[:, 0, :]
                    proj("wk", xc, xcl, ps_k)
                    rot_path(ps_k, csl, krot[:, sl], "k")

                def b_pf():
                    out[0] = load_chunk(nxt) if nxt is not None else None

                return [b_q, b_v(0), b_v(2), b_k, b_pf]

            def emit_exp(scp2, i, j, qlo):
                """one op over BOTH heads' [128, 2, SQC-qlo] scores.
                returns (ex2_tile, kind): kind 'f8' or 'f16'"""
                diag = causal and i >= 4 * j
                if causal or mode == "zeros":
                    if diag:
                        ex = exp_p.tile([128, 2, SQC], F16, tag="ex16", name="ex16")
                        g = (
                            gd_sb[:, i - 4 * j, qlo:]
                            .unsqueeze(1)
                            .to_broadcast((128, 2, SQC - qlo))
                        )
                        nc.vector.scalar_tensor_tensor(
                            ex[:, :, qlo:].bitcast(I16),
                            scp2[:, :, qlo:], EBIAS, g,
                            AluOpType.add, AluOpType.min,
                        )
                        return ex, "f16"
                    ex = exp_p.tile([128, 2, SQC], F8E4, tag="ex8", name="ex8")
                    w = o["exp_w"] if j < o["exp_late"] else o["exp_w2"]
                    eng = pick("exp", w, (nc.scalar, nc.vector))
                    if eng is nc.scalar:
                        nc.scalar.activation(
                            ex[:], scp2[:], AF.Exp, bias=biast8[:], scale=ACT_SCALE
                        )
                    else:
                        nc.vector.tensor_scalar(
                            ex[:].bitcast(U8), scp2[:], 1.0 / 128.0, BIAS8,
                            AluOpType.mult, AluOpType.add,
                        )
                    return ex, "f8"
                # general: additive mask (same rows for both heads), fp16 trick
                mt = mkp.tile([128, SQC], F32, tag="mk", name="mt")
                nc.sync.dma_start(mt[:], mask_d[j, i * 128 : (i + 1) * 128, :])
                t = mkp.tile([128, 2, SQC], F32, tag="mks", name="mts")
                nc.vector.tensor_tensor(
                    t[:], scp2[:],
                    mt[:].unsqueeze(1).to_broadcast((128, 2, SQC)),
                    op=AluOpType.add,
                )
                ex = exp_p.tile([128, 2, SQC], F16, tag="ex16", name="ex16")
                t2 = mkp.tile([128, 2, SQC], F32, tag="mks2", name="mts2")
                nc.vector.tensor_scalar(
                    t2[:], t[:], EBIAS, ECLAMP, AluOpType.add, AluOpType.min
                )
                nc.vector.tensor_scalar_max(ex[:].bitcast(I16), t2[:], 0.0)
                return ex, "f16"

            def do_attn_chunk(j, oproj_after=None, pblocks=None):
                sl = slice(j * SQC, (j + 1) * SQC)
                nkt_j = 4 * (j + 1) if causal else NKT
                at2 = atp.tile([128, 2, SQC], F32, tag="at", name="at2")
                pend = []

                def emit_scores(i):
                    # diagonal tiles only have valid queries n >= (i-4j)*128
                    qlo = (i - 4 * j) * 128 if (causal and i > 4 * j) else 0
                    scp2 = scp_p.tile([128, 2, SQC], F32, tag="scp", name="scp")
                    for h in range(HPC):
                        hsl = slice(h * 64, (h + 1) * 64)
                        nc.tensor.matmul(
                            scp2[:, h, qlo:],
                            krot[hsl, i * 128 : (i + 1) * 128],
                            qrot_all[hsl, j * SQC + qlo : (j + 1) * SQC],
                            start=True, stop=True,
                            skip_group_check=True,
                        )
                    ex, kind = emit_exp(scp2, i, j, qlo)
                    return ex, qlo, kind

                def emit_attnv(i, ex2, qlo, kind, first, last):
                    for h in range(HPC):
                        out_ap = at2[0:65, h, qlo:]
                        if kind == "f8":
                            rhs = (
                                ex2[:, h, :]
                                .unsqueeze(1)
                                .to_broadcast((128, 2, SQC))
                            )
                            nc.tensor.matmul(
                                out_ap,
                                vext8[:, i, h, :, 0:65],
                                rhs,
                                start=first, stop=last,
                                perf_mode=DRMODE,
                                skip_group_check=True,
                            )
                        else:
                            nc.tensor.matmul(
                                out_ap,
                                vext16[:, i, h, 0:65],
                                ex2[:, h, qlo:],
                                start=first, stop=last,
                                skip_group_check=True,
                            )

                # tile order: spread the diagonal tiles (whose exp is
                # DVE-only) evenly among non-diag tiles so the two exp
                # engines stay co-busy through the whole chunk
                if causal:
                    m = 4 * j
                    dslots = {((k + 1) * (m + 4)) // 5: k for k in range(4)}
                    order = []
                    ni = di = 0
                    for s in range(m + 4):
                        if (s in dslots and di <= dslots[s]) or ni >= m:
                            order.append(m + di)
                            di += 1
                        else:
                            order.append(ni)
                            ni += 1
                    while di < 4:
                        order.append(m + di)
                        di += 1
                    assert sorted(order) == list(range(nkt_j)), order
                else:
                    order = list(range(nkt_j))

                depth = o["swpipe"]
                hooks = list(oproj_after or [])
                blocks = list(pblocks or [])
                bl_emitted = 0
                op_emitted = 0
                op_total = len(hooks) * (KT // 2)
                be = o["bl_every"]
                for n_e, i in enumerate(order):
                    ex2, qlo, kind = emit_scores(i)
                    pend.append((i, ex2, qlo, kind, n_e == 0, n_e == nkt_j - 1))
                    # next chunk's projection sub-blocks and deferred o-proj
                    # pairs land spread through this chunk's score stream
                    if n_e % be == 0 and bl_emitted < len(blocks):
                        blocks[bl_emitted]()
                        bl_emitted += 1
                    elif op_emitted < op_total and n_e % 2 == 1:
                        hooks[op_emitted // (KT // 2)][0](op_emitted % (KT // 2))
                        op_emitted += 1
                    if len(pend) > depth:
                        emit_attnv(*pend.pop(0))
                while bl_emitted < len(blocks):
                    blocks[bl_emitted]()
                    bl_emitted += 1
                while op_emitted < op_total:
                    hooks[op_emitted // (KT // 2)][0](op_emitted % (KT // 2))
                    op_emitted += 1
                for hk in hooks:
                    hk[1]()
                for p in pend:
                    emit_attnv(*p)

                # ONE reciprocal over both heads' denominator rows, then DMA
                # partition-broadcasts (stride-0 free-dim read replicates the
                # one-partition row across 64 partitions).
                rec = rcp.tile([65, 2, SQC], F32, tag="rec", name="rec")
                with nc.allow_low_precision("reciprocal of softmax denom"):
                    nc.vector.reciprocal(rec[64:65, :, :], at2[64:65, :, :])
                recbs = []
                for h in range(HPC):
                    recb = rbp.tile([64, SQC], F32, tag=f"recb{h}", name=f"recb{h}")
                    nc.sync.dma_start(
                        recb[:],
                        rec[64:65, h, :].unsqueeze(1).to_broadcast((1, 64, SQC)),
                    )
                    recbs.append(recb)

                def norm_part():
                    # normalize directly from attnV PSUM against the DMA-
                    # broadcast reciprocal rows; h1 first so its partition-
                    # move DMA overlaps h0's work
                    afin = afp.tile([128, SQC], F16, tag="afin", name="afin")
                    tmph = afp.tile([64, SQC], F16, tag="tmph", name="tmph")
                    nc.vector.tensor_tensor(
                        tmph[:], at2[0:64, 1, :], recbs[1][:],
                        op=AluOpType.mult,
                    )
                    nc.sync.dma_start(afin[64:128, :], tmph[:])
                    nc.vector.tensor_tensor(
                        afin[0:64, :], at2[0:64, 0, :], recbs[0][:],
                        op=AluOpType.mult,
                    )
                    return afin

                def oproj_pairs(afin):
                    """generator of per-pair emitters: the o-proj matmuls
                    write into freed at-pool pair tiles, spread through the
                    next chunk's score stream so copies finish between."""
                    os_big = oop.tile([128, KT, SQC], BF16, tag="oo", name="os_big")

                    def one(dp, drain=False):
                        op2 = scp_p.tile([128, 2, SQC], F32, tag="scp", name="op2")
                        for g in range(2):
                            dt_i = 2 * dp + g
                            nc.tensor.matmul(
                                op2[:, g, :],
                                wo_sb[:, dt_i * 128 : (dt_i + 1) * 128],
                                afin[:],
                                start=True, stop=True,
                                skip_group_check=True,
                            )
                        if drain:
                            eng = nc.vector if dp % 2 == 0 else nc.scalar
                        else:
                            eng = pick("cp", o["cp_w"], (nc.vector, nc.scalar))
                        if eng is nc.scalar:
                            nc.scalar.copy(
                                os_big[:, 2 * dp : 2 * dp + 2, :], op2[:]
                            )
                        else:
                            eng.tensor_copy(
                                os_big[:, 2 * dp : 2 * dp + 2, :], op2[:]
                            )

                    def flush(drain=False):
                        if drain:
                            # two halves: fewer serial DMA headers than
                            # per-tile, still overlaps the last copies
                            for k0 in (0, KT // 2):
                                nc.sync.dma_start(
                                    out_d[
                                        k0 * 128 : (k0 + KT // 2) * 128, sl
                                    ].rearrange("(k p) n -> p k n", p=128),
                                    os_big[:, k0 : k0 + KT // 2, :],
                                )
                        else:
                            nc.sync.dma_start(
                                out_d[:, sl].rearrange("(k p) n -> p k n", p=128),
                                os_big[:],
                            )

                    return one, flush

                return (norm_part, oproj_pairs)

            # Projections run one chunk AHEAD of attention: proj(j) then
            # attn(j-1). The x-load DMAs stream behind attention compute,
            # and attention's exp-engine load overlaps projection's PE burst.
            # Chunk a's normalize runs at the start of attn(a+1); its o-proj
            # pairs spread through attn(a+2)'s score stream.
            pend_out = []
            state = {"pend_norm": None, "ready_op": None}

            def attn_step(aj, pblocks=None, last=False):
                hooks = []
                if state["ready_op"] is not None:
                    hooks.append(state["ready_op"])
                    state["ready_op"] = None
                if state["pend_norm"] is not None:
                    np_, opg_ = state["pend_norm"]
                    afin_prev = np_()
                    state["ready_op"] = opg_(afin_prev)
                    state["pend_norm"] = None
                if last and state["ready_op"] is not None:
                    hooks.append(state["ready_op"])
                    state["ready_op"] = None
                state["pend_norm"] = do_attn_chunk(
                    aj, oproj_after=hooks or None, pblocks=pblocks
                )

            loaded = (xc0, xcl0, csl0)
            loaded_nxt = load_chunk(1)  # 2-deep x prefetch (xc bufs=3)
            for j in range(NJ):
                nxt = j + 2 if j + 2 < NJ else None
                pf_out = [None]
                blocks = proj_blocks(j, loaded, nxt, pf_out)
                if o["fine_ilv"] and j >= 1:
                    attn_step(j - 1, pblocks=blocks)
                else:
                    for b in blocks:
                        b()
                    if j >= 1:
                        attn_step(j - 1)
                loaded = loaded_nxt
                loaded_nxt = pf_out[0]
            attn_step(NJ - 1, last=True)
            # tail: the last chunk's normalize + o-proj drain
            np_, opg_ = state["pend_norm"]
            afin_last = np_()
            one, flush = opg_(afin_last)
            for dp in range(KT // 2):
                one(dp, drain=True)
            flush(drain=True)

    _fix_waits(nc)
    return nc


def _host_prep(x, cos, sin, mask, wq, wk, wv, wo):
    x = np.asarray(x, dtype=np.float32)
    cos = np.asarray(cos, dtype=np.float32)
    sin = np.asarray(sin, dtype=np.float32)
    mask = np.asarray(mask, dtype=np.float32)
    wq = np.asarray(wq, dtype=np.float32)
    wk = np.asarray(wk, dtype=np.float32)
    wv = np.asarray(wv, dtype=np.float32)
    wo = np.asarray(wo, dtype=np.float32)
    import ml_dtypes

    E4 = ml_dtypes.float8_e4m3
    E5 = ml_dtypes.float8_e5m2
    xT = np.ascontiguousarray(x.reshape(S, D).T)
    xh = xT.astype(E4)
    xl = (xT - xh.astype(np.float32)).astype(E5)

    # cos/sin rows: partition p -> rotation pair (p % 64)//2; sign on sin
    idx = np.repeat(np.arange(HD // 2), 2)                 # (64,)
    cosr = cos[:, idx].T                                   # (64, S)
    sinr = sin[:, idx].T
    sgn = np.where(np.arange(HD) % 2 == 0, -1.0, 1.0)[:, None]
    csc = CS / 32.0                       # undo the 32x fp8 weight rescale
    cos128 = np.vstack([cosr, cosr]) * csc
    sin128 = np.vstack([sinr * sgn, sinr * sgn]) * csc
    # sin''[p] = sin'[p^1]: the mul happens BEFORE the partition shift
    sin128 = sin128.reshape(64, 2, S)[:, ::-1, :].reshape(128, S)
    cossin = np.concatenate(
        [cos128[:, None, :], sin128[:, None, :]], axis=1
    ).reshape(128, 2 * S).astype(np.float16)

    neg = np.isneginf(mask)
    triu = np.triu(np.ones((S, S), dtype=bool), 1)
    if not neg.any() and not mask.any():
        mode = "zeros"
    elif np.array_equal(neg, triu) and not mask[~neg].any():
        mode = "causal"
        blk0 = mask[0:SQC, 0:SQC]
        # gate[p, r, n] over score tile [sk=128, sq=512]: valid iff not -inf
        gd = np.empty((128, 4, SQC), np.float32)
        for r in range(4):
            blkv = ~np.isneginf(blk0[:, r * 128 : (r + 1) * 128])  # (q, k)
            gd[:, r, :] = np.where(blkv.T, ECLAMP, 0.0)
        gdiag = np.ascontiguousarray(gd.reshape(128, 4 * SQC)).astype(np.float16)
    else:
        mode = "general"
        maskT = np.empty((NJ, S, SQC), np.float32)
        for j in range(NJ):
            maskT[j] = mask[j * SQC : (j + 1) * SQC, :].T * np.float32(L2E1024)

    per_core = []
    for c in range(NC):
        hs, he = c * 128, (c + 1) * 128
        m = {"xh": xh, "xl": xl, "cossin": cossin}
        # lhsT weight layout: w_t[p, k*128+m] = 32 * w_slice[m, k*128+p]
        for name, w in (("wq", wq), ("wk", wk), ("wv", wv)):
            ws = w[hs:he, :] * np.float32(32.0)            # (128, D)
            t = ws.T.reshape(KT, 128, 128)                 # (k, p, m)
            wt = np.ascontiguousarray(t.transpose(1, 0, 2).reshape(128, D))
            whq = wt.astype(E4)
            m[name + "h"] = whq
            m[name + "l"] = (wt - whq.astype(np.float32)).astype(E5)
        m["wo"] = np.ascontiguousarray(wo[:, hs:he].T).astype(np.float16)
        if mode == "causal":
            m["gdiag"] = gdiag
        elif mode == "general":
            m["maskT"] = maskT
        per_core.append(m)
    return mode, per_core


_cache = {}


def kernel(x, cos, sin, mask, wq, wk, wv, wo, start_pos=0, **_):
    mode, in_maps = _host_prep(x, cos, sin, mask, wq, wk, wv, wo)
    if mode not in _cache:
        _cache[mode] = build_program(mode)
    nc = _cache[mode]
    res = run_bass_kernel_spmd(nc, in_maps, core_ids=list(range(NC)))
    acc = np.zeros((D, S), np.float64)
    for c in range(NC):
        acc += res.results[c]["opT"].astype(np.float64)
    return np.ascontiguousarray(acc.T).reshape(B, S, D).astype(np.float32)
